# revision 11
# baseline (speedup 1.0000x reference)
"""AttentionBlock (GroupNorm + single-head self-attention + residual) on 8 TRN2 cores.

Data-parallel over batch: 32 samples -> 4 per core; weights replicated.

fp8e4 DoubleRow matmuls (2 K-tiles of 128 contracted per instruction, 2x PE
throughput vs bf16). All operands stored K-paired: tile j holds K blocks
2j (sub 0) and 2j+1 (sub 1) as [128, 2, free].

Weights are scaled x64 on host (raw weights ~U(-0.044, 0.044) sit in fp8's
subnormal range); the 1/64 descale folds into existing evict ops. The
fused rowsum matmul uses an all-(1/64) stationary so every psum partition
receives rowsum/64 (rowsum + broadcast + descale in one step); oT is then
~unit-scale in fp8 and the projection evict descales by 1/4096. bp (+
Wp@bv) is folded into the residual shift column (shbp).

Software pipeline (engine queues are in-order, so emission order is
execution order): sample s+1's Q/K/V matmuls are emitted between sample
s's scores and attention, hiding the ACT Exp latency; x is DMA'd once and
cached in SBUF (stats + xn read the same tiles); stats/affine for sample
s+2 are prefetched a round ahead.

Head choreography (the DMA rings saturate at ~300 GB/s and process
descriptors roughly FIFO): x(0)'s triggers are emitted first from three
queues; weights wait on a Pool-side barrier read of x(0), and x(1) is
triggered on the Pool queue behind the weights, so the ring order is
strictly x(0) | weights | x(1). Meanwhile PE runs a chain of warm-up
matmuls on dummy data: they fill the otherwise-idle head and ramp the PE
p-state (a PE gap drops the clock to half for the next ~3us) so the first
real matmuls run at full speed.
"""

import sys

if "/opt/trn_rl_repo" not in sys.path:
    sys.path.insert(0, "/opt/trn_rl_repo")

from contextlib import ExitStack

import numpy as np

import concourse.bass as bass
import concourse.tile as tile
from concourse import bacc, mybir
from concourse.bass_utils import run_bass_kernel_spmd

N_CORES = 8
B, C, H, W = 32, 512, 32, 32
HW = H * W            # tokens per sample (N)
SPC = B // N_CORES    # samples per core
G = 8                 # groups
GSZ = C // G          # channels per group (64)
EPS = 1e-5
P = 128               # partitions
CT = C // P           # channel tiles (4)
NT = HW // P          # token tiles (8)
NCHUNK = HW // 512    # 512-wide free-dim chunks over tokens (2)
SCALE = C ** -0.5
WS = 64.0             # host-side weight scale (fp8 range)
IWS = 1.0 / WS
N_WARMUP = 42         # PE warm-up matmuls during the head

F32 = mybir.dt.float32
FP8 = mybir.dt.float8e4
AF = mybir.ActivationFunctionType
ALU = mybir.AluOpType
DR = mybir.MatmulPerfMode.DoubleRow

FREE = 512   # moving-operand width per matmul (PSUM one-bank limit)
NJC = HW // FREE


def _declare_io(nc):
    def inp(name, shape, dt=F32):
        return nc.dram_tensor(name, list(shape), dt, kind="ExternalInput").ap()

    aps = {
        "x": inp("x", (SPC, C, HW)),
        "wq8": inp("wq8", (P, 2, 2, C), FP8),
        "wk8": inp("wk8", (P, 2, 2, C), FP8),
        "wv8": inp("wv8", (P, 2, 2, C), FP8),
        "wp8": inp("wp8", (P, 2, 2, C), FP8),
        "smalls": inp("smalls", (P, 23)),
        "member_t": inp("member_t", (2, P)),
        "out": nc.dram_tensor("out", [SPC, C, HW], F32, kind="ExternalOutput").ap(),
    }
    return aps


def _build_tile_kernel(ctx: ExitStack, tc: tile.TileContext, aps):
    nc = tc.nc

    singles = ctx.enter_context(tc.tile_pool(name="singles", bufs=1))
    wpool = ctx.enter_context(tc.tile_pool(name="wpool", bufs=1))
    xpool = ctx.enter_context(tc.tile_pool(name="xpool", bufs=2))
    xnpool = ctx.enter_context(tc.tile_pool(name="xnpool", bufs=2))
    qpool = ctx.enter_context(tc.tile_pool(name="qpool", bufs=2))
    kpool = ctx.enter_context(tc.tile_pool(name="kpool", bufs=2))
    vpool = ctx.enter_context(tc.tile_pool(name="vpool", bufs=2))
    epool = ctx.enter_context(tc.tile_pool(name="epool", bufs=2))
    opool = ctx.enter_context(tc.tile_pool(name="opool", bufs=2))
    stat = ctx.enter_context(tc.tile_pool(name="stat", bufs=2))
    sall = ctx.enter_context(tc.tile_pool(name="sall", bufs=1))
    rpool = ctx.enter_context(tc.tile_pool(name="rpool", bufs=2))
    respool = ctx.enter_context(tc.tile_pool(name="respool", bufs=5))

    psum_big = ctx.enter_context(tc.tile_pool(name="psum_big", bufs=3, space="PSUM"))
    psum_sm = ctx.enter_context(tc.tile_pool(name="psum_sm", bufs=2, space="PSUM"))

    xtiles = {}   # (s, ct) -> cached x tile (loaded once, reused by xn)

    def x_dma(s, spread=False, eng=None):
        # spread=True issues the 8 half-DMAs from 3 different engine queues
        # so descriptor generation (~650ns per trigger) parallelizes -- used
        # for sample 0 where the x load is the critical path. eng pins all
        # triggers to one queue (used to sequence x(1) behind the weights on
        # the Pool queue).
        # full-tile DMAs: 4KB descriptors halve ring occupancy vs 2KB halves
        engs = ([nc.sync, nc.scalar, nc.gpsimd] if spread
                else [eng or nc.sync])
        for ct in range(CT):
            xt = xpool.tile([P, HW], F32, name=f"x_{s}_{ct}", tag=f"x{ct}")
            engs[ct % len(engs)].dma_start(
                out=xt[:], in_=aps["x"][s, ct * P:(ct + 1) * P, :])
            xtiles[(s, ct)] = xt

    # ---- head: x(0) descriptors hit the rings before anything else ----
    x_dma(0, spread=True)

    # small constants (scalar queue; lands right behind x(0)'s first rows)
    smalls = singles.tile([P, 23], F32, tag="smalls")
    nc.scalar.dma_start(out=smalls[:], in_=aps["smalls"][:])
    bq_c = smalls[:, 0:CT]
    bk_c = smalls[:, CT:2 * CT]
    bp_c = smalls[:, 2 * CT:3 * CT]
    gamma_c = smalls[:, 3 * CT:4 * CT]
    beta_c = smalls[:, 4 * CT:5 * CT]
    member = smalls[:, 20:22]

    member_t = singles.tile([2, P], F32, tag="member_t")
    nc.scalar.dma_start(out=member_t[:], in_=aps["member_t"][:])

    # fp8 (1/WS) stationary for the fused rowsum+broadcast DR matmul, plus
    # a 512-wide dummy moving tile for the PE warm-up chain.
    ones_stage = singles.tile([P, HW], F32, tag="ones_stage")
    nc.vector.memset(ones_stage[:], 1.0 / WS)
    ones2 = singles.tile([P, 2, P], FP8, tag="ones2")
    nc.vector.tensor_copy(out=ones2[:].rearrange("p i m -> p (i m)"),
                          in_=ones_stage[:, 0:2 * P])
    dm8 = singles.tile([P, 2, 512], FP8, tag="dm8")
    nc.vector.tensor_copy(out=dm8[:].rearrange("p i m -> p (i m)"),
                          in_=ones_stage[:])

    I32 = mybir.dt.int32
    magic = singles.tile([2, SPC * CT], I32, tag="magic")
    nc.vector.memset(magic[:], 0x5F3759DF)

    # prime the ACT Exp spline table before any real dependency needs it
    warm = singles.tile([1, 1], F32, tag="warm")
    nc.vector.memset(warm[:], 1.0)
    nc.scalar.activation(out=warm[:], in_=warm[:], func=AF.Exp)

    # PE warm-up: fills the DMA/stats-bound head and ramps the PE p-state
    # so the first real matmuls run at full clock.
    for i in range(N_WARMUP):
        wps = psum_big.tile([P, 512], F32, tag="big")
        nc.tensor.matmul(wps[:], ones2[:], dm8[:], start=True, stop=True,
                         perf_mode=DR)

    # ---- weights: Pool-queue barrier behind x(0), then x(1) behind them ----
    wr = {}

    def load_weights(barrier_tile):
        if barrier_tile is not None:
            dummy = singles.tile([P, 1], F32, tag="wbar")
            nc.gpsimd.tensor_copy(out=dummy[:], in_=barrier_tile[:, 1023:1024])
        for wname in ("wq8", "wk8", "wv8", "wp8"):
            t = wpool.tile([P, 2, 2, C], FP8, name=f"{wname}t", tag=wname)
            nc.gpsimd.dma_start(out=t[:], in_=aps[wname][:])
            wr[wname] = t

    load_weights(xtiles[(0, 3)])
    x_dma(1, eng=nc.gpsimd)

    def w_sl(wname, j, blk):
        return wr[wname][:, j, :, blk * P:(blk + 1) * P]

    # ======== GroupNorm statistics ========
    stats_all = sall.tile([2, SPC, CT, 2], F32, tag="stats_all")
    sc = sall.tile([P, SPC, CT], F32, tag="sc")
    sh = sall.tile([P, SPC, CT], F32, tag="sh")
    shbp = sall.tile([P, SPC, CT], F32, tag="shbp")

    def gn_stats(s, pe_after=None, dve_after=None):
        # Anchors pin this block behind the given instructions in the static
        # engine queues -- the build-time scheduler otherwise hoists it into
        # earlier PE/DVE slots (its DMA model is optimistic about x's
        # arrival) and the real hardware stalls there.
        partials = stat.tile([P, CT, 2], F32, tag="partials")
        for ct in range(CT):
            xt = xtiles[(s, ct)]
            st6 = stat.tile([P, 2, 6], F32, tag="st6")
            d = nc.vector.bn_stats(out=st6[:, 0, :], in_=xt[:, 0:512])
            if dve_after is not None:
                tile.add_dep_helper(d.ins, dve_after.ins,
                                    reason="stats yield DVE to evicts")
                dve_after = None
            nc.vector.bn_stats(out=st6[:, 1, :], in_=xt[:, 512:1024])
            nc.vector.bn_aggr(out=partials[:, ct, :], in_=st6[:])
            nc.vector.scalar_tensor_tensor(
                out=partials[:, ct, 1:2], in0=partials[:, ct, 0:1],
                scalar=partials[:, ct, 0:1], in1=partials[:, ct, 1:2],
                op0=ALU.mult, op1=ALU.add)
        ps = psum_sm.tile([2, CT * 2], F32, tag="sm")
        m = nc.tensor.matmul(ps[:], member[:],
                             partials[:].rearrange("p t j -> p (t j)"),
                             start=True, stop=True)
        if pe_after is not None:
            tile.add_dep_helper(m.ins, pe_after.ins,
                                reason="stats-mm after attnV stream")
        nc.vector.tensor_copy(out=stats_all[:, s, :, :],
                              in_=ps[:].rearrange("p (t j) -> p t j", j=2))

    def gn_affine(s0, ns):
        mv = stats_all[:, s0:s0 + ns, :, 0]
        sv = stats_all[:, s0:s0 + ns, :, 1]
        msq = stat.tile([2, ns, CT], F32, tag="msq")
        nc.vector.tensor_mul(out=msq[:], in0=mv, in1=mv)
        nc.vector.tensor_sub(out=sv, in0=sv, in1=msq[:])
        # rstd = rsqrt(var + eps) on DVE: bit-trick seed + 3 Newton iters.
        vadd = stat.tile([2, ns, CT], F32, tag="vadd")
        nc.vector.tensor_scalar_add(out=vadd[:], in0=sv, scalar1=float(EPS))
        z = stat.tile([2, ns, CT], F32, tag="z")
        z_i = z[:].bitcast(I32)
        nc.vector.tensor_scalar(out=z_i, in0=vadd[:].bitcast(I32), scalar1=1,
                                scalar2=None, op0=ALU.arith_shift_right)
        mg = magic[:, 0:ns * CT].rearrange("p (s t) -> p s t", t=CT)
        nc.vector.scalar_tensor_tensor(out=z_i, in0=mg, scalar=0, in1=z_i,
                                       op0=ALU.bypass, op1=ALU.subtract)
        # 2 Newton iterations: bit-trick seed err ~3.4% -> 0.17% -> 4e-6,
        # negligible vs the fp8 path's ~1e-3.
        nt_ = stat.tile([2, ns, CT], F32, tag="nt")
        for _ in range(2):
            nc.vector.tensor_mul(out=nt_[:], in0=z[:], in1=z[:])
            nc.vector.tensor_mul(out=nt_[:], in0=nt_[:], in1=vadd[:])
            nc.vector.tensor_scalar(out=nt_[:], in0=nt_[:], scalar1=-0.5,
                                    scalar2=1.5, op0=ALU.mult, op1=ALU.add)
            nc.vector.tensor_mul(out=z[:], in0=z[:], in1=nt_[:])
        nc.vector.tensor_copy(out=sv, in_=z[:])
        ab = stat.tile([2, ns, CT, 2], F32, tag="ab")
        nc.vector.tensor_copy(out=ab[:, :, :, 0], in_=sv)
        nc.vector.scalar_tensor_tensor(out=ab[:, :, :, 1], in0=mv, scalar=-1.0,
                                       in1=sv, op0=ALU.mult, op1=ALU.mult)
        sb_ps = psum_sm.tile([P, ns * CT * 2], F32, tag="sm")
        sb_mm = nc.tensor.matmul(sb_ps[:], member_t[:],
                                 ab[:].rearrange("p s t j -> p (s t j)"),
                                 start=True, stop=True)
        sb = stat.tile([P, ns, CT, 2], F32, tag="sb")
        nc.vector.tensor_copy(
            out=sb[:], in_=sb_ps[:].rearrange("p (s t j) -> p s t j", t=CT, j=2))
        for i in range(ns):
            s = s0 + i
            for ct in range(CT):
                nc.vector.tensor_scalar_mul(out=sc[:, s, ct:ct + 1],
                                            in0=gamma_c[:, ct:ct + 1],
                                            scalar1=sb[:, i, ct, 0:1])
                nc.vector.scalar_tensor_tensor(out=sh[:, s, ct:ct + 1],
                                               in0=gamma_c[:, ct:ct + 1],
                                               scalar=sb[:, i, ct, 1:2],
                                               in1=beta_c[:, ct:ct + 1],
                                               op0=ALU.mult, op1=ALU.add)
                nc.vector.tensor_tensor(out=shbp[:, s, ct:ct + 1],
                                        in0=sh[:, s, ct:ct + 1],
                                        in1=bp_c[:, ct:ct + 1], op=ALU.add)
        return sb_mm

    # ======== per-sample phase emitters ========
    xn8_all = {}
    xnf_all = {}
    q8_all = {}
    k8_all = {}
    vT8_all = {}
    eT8_all = {}
    rb_all = {}
    oT8_all = {}

    def xn_make(s, eng=None, skip_xnf=False):
        # Pool (SBUF->SBUF): fp8 matmul input first (feeds PE), then the
        # f32 residual (only needed at the projection evict). Sample 0 uses
        # DVE (faster) since the head's first matmuls wait on it.
        eng = eng or nc.gpsimd
        xn8 = [xnpool.tile([P, 2, HW], FP8, name=f"xn8_{s}_{j}", tag=f"xn8{j}")
               for j in range(2)]
        for ct in range(CT):
            eng.tensor_scalar(out=xn8[ct // 2][:, ct % 2, :],
                              in0=xtiles[(s, ct)][:],
                              scalar1=sc[:, s, ct:ct + 1],
                              scalar2=sh[:, s, ct:ct + 1],
                              op0=ALU.mult, op1=ALU.add)
        xn8_all[s] = xn8
        if not skip_xnf:
            xnf_make(s)

    def xnf_make(s):
        xnf = []
        for ct in range(CT):
            tf = xnpool.tile([P, HW], F32, name=f"xnf_{s}_{ct}", tag=f"xnf{ct}")
            nc.gpsimd.tensor_scalar(out=tf[:], in0=xtiles[(s, ct)][:],
                                    scalar1=sc[:, s, ct:ct + 1],
                                    scalar2=shbp[:, s, ct:ct + 1],
                                    op0=ALU.mult, op1=ALU.add)
            xnf.append(tf)
        xnf_all[s] = xnf

    def qk_phase(s):
        xn8 = xn8_all[s]
        for pname, bcol, pool_, store in (
                ("q", bq_c, qpool, q8_all), ("k", bk_c, kpool, k8_all)):
            wname = "wq8" if pname == "q" else "wk8"
            tiles = [pool_.tile([P, 2, HW], FP8, name=f"{pname}8_{s}_{j}",
                                tag=f"{pname}{j}") for j in range(2)]
            for dt in range(CT):
                ps = psum_big.tile([P, HW], F32, tag="big")
                for jc in range(NJC):
                    for j in range(2):
                        nc.tensor.matmul(ps[:, jc * FREE:(jc + 1) * FREE],
                                         w_sl(wname, j, dt),
                                         xn8[j][:, :, jc * FREE:(jc + 1) * FREE],
                                         start=(j == 0), stop=(j == 1),
                                         perf_mode=DR)
                nc.scalar.activation(out=tiles[dt // 2][:, dt % 2, :], in_=ps[:],
                                     func=AF.Identity, bias=bcol[:, dt:dt + 1],
                                     scale=IWS)
            store[s] = tiles

    def v_phase(s, hold=0):
        # hold>0 defers the last `hold` token-groups to v_phase_tail -- used
        # as PE filler between attnV(s-1) and proj(s-1) to cover the
        # trailing oT evict latency.
        xn8 = xn8_all[s]
        vT8 = [vpool.tile([P, 2, C], FP8, name=f"vT8_{s}_{j}", tag=f"v{j}")
               for j in range(4)]
        vT8_all[s] = vT8
        for nt in range(NT - hold):
            _v_group(s, nt)

    def _v_group(s, nt):
        xn8, vT8 = xn8_all[s], vT8_all[s]
        ps_full = psum_big.tile([P, HW], F32, tag="big")
        ps = ps_full[:, 0:512]
        for j in range(2):
            nc.tensor.matmul(ps[:], xn8[j][:, :, nt * P:(nt + 1) * P],
                             wr["wv8"][:, j],
                             start=(j == 0), stop=(j == 1), perf_mode=DR)
        nc.scalar.activation(out=vT8[nt // 2][:, nt % 2, :], in_=ps[:],
                             func=AF.Identity, scale=IWS)

    def v_phase_tail(s, hold):
        for nt in range(NT - hold, NT):
            _v_group(s, nt)

    def scores_phase(s, split_exp=False):
        q8, k8 = q8_all[s], k8_all[s]
        eT8 = [epool.tile([P, 2, HW], FP8, name=f"eT8_{s}_{j}", tag=f"e{j}")
               for j in range(4)]
        pss = []
        for mt in range(NT):
            ps = psum_big.tile([P, HW], F32, tag="big")
            for jc in range(NJC):
                for j in range(2):
                    nc.tensor.matmul(ps[:, jc * FREE:(jc + 1) * FREE],
                                     k8[j][:, :, mt * P:(mt + 1) * P],
                                     q8[j][:, :, jc * FREE:(jc + 1) * FREE],
                                     start=(j == 0), stop=(j == 1), perf_mode=DR)
            if split_exp:
                # jc0-half exps first so the tail's rowsum/attnV jc0 groups
                # start ~5us earlier (used for the last sample).
                pss.append(ps)
                nc.scalar.activation(out=eT8[mt // 2][:, mt % 2, 0:512],
                                     in_=ps[:, 0:512], func=AF.Exp, scale=SCALE)
            else:
                nc.scalar.activation(out=eT8[mt // 2][:, mt % 2, :], in_=ps[:],
                                     func=AF.Exp, scale=SCALE)
        if split_exp:
            for mt in range(NT):
                nc.scalar.activation(out=eT8[mt // 2][:, mt % 2, 512:1024],
                                     in_=pss[mt][:, 512:1024], func=AF.Exp,
                                     scale=SCALE)
        eT8_all[s] = eT8

    def softmax_phase(s):
        # fused rowsum + broadcast + 1/WS: rb = WS/rowsum on every partition
        eT8 = eT8_all[s]
        rb = rpool.tile([P, HW], F32, name=f"rb_{s}", tag="rb")
        for jc in range(NCHUNK):
            rs_ps = psum_sm.tile([P, 512], F32, tag="sm")
            for j4 in range(4):
                nc.tensor.matmul(rs_ps[:], ones2[:],
                                 eT8[j4][:, :, jc * 512:(jc + 1) * 512],
                                 start=(j4 == 0), stop=(j4 == 3), perf_mode=DR)
            nc.vector.reciprocal_approx_fast(out=rb[:, jc * 512:(jc + 1) * 512],
                                             in_=rs_ps[:])
        rb_all[s] = rb

    def attnv_phase(s):
        eT8, vT8, rb = eT8_all[s], vT8_all[s], rb_all[s]
        oT8 = [opool.tile([P, 2, HW], FP8, name=f"oT8_{s}_{j}", tag=f"o{j}")
               for j in range(2)]
        # jc-outer: all jc0 evicts land first, so the projection's jc0
        # groups can run while the jc1 half is still evicting.
        last_mm = last_ev = None
        for jc in range(NCHUNK):
            for dt in range(CT):
                ps = psum_big.tile([P, 512], F32, tag="big")
                for j4 in range(4):
                    last_mm = nc.tensor.matmul(
                        ps[:], vT8[j4][:, :, dt * P:(dt + 1) * P],
                        eT8[j4][:, :, jc * 512:(jc + 1) * 512],
                        start=(j4 == 0), stop=(j4 == 3), perf_mode=DR)
                last_ev = nc.vector.tensor_mul(
                    out=oT8[dt // 2][:, dt % 2, jc * 512:(jc + 1) * 512],
                    in0=ps[:], in1=rb[:, jc * 512:(jc + 1) * 512])
        oT8_all[s] = oT8
        return last_mm, last_ev

    def proj_phase(s):
        # jc-outer to pair with attnv_phase: the jc0 projection only needs
        # the jc0 oT evicts (first half of attnV's evict stream).
        oT8, xnf = oT8_all[s], xnf_all[s]
        res_t = [respool.tile([P, HW], F32, name=f"res_{s}_{et}", tag="res")
                 for et in range(CT)]
        for jc in range(NJC):
            sl = slice(jc * FREE, (jc + 1) * FREE)
            for et in range(CT):
                ps = psum_big.tile([P, FREE], F32, tag="big")
                for j in range(2):
                    nc.tensor.matmul(ps[:], w_sl("wp8", j, et),
                                     oT8[j][:, :, sl],
                                     start=(j == 0), stop=(j == 1), perf_mode=DR)
                nc.vector.scalar_tensor_tensor(
                    out=res_t[et][:, sl], in0=ps[:], scalar=1.0 / (WS * WS),
                    in1=xnf[et][:, sl], op0=ALU.mult, op1=ALU.add)
                nc.sync.dma_start(out=aps["out"][s, et * P:(et + 1) * P, sl],
                                  in_=res_t[et][:, sl])

    # ======== schedule ========
    # The head's serial chain (x(0) -> stats -> affine -> xn8) gates the
    # first real matmul; high_priority keeps the build-time scheduler from
    # diluting it with later-emitted ready work (its DMA model is
    # optimistic, so sample-1 stats often look "ready" too early).
    with tc.high_priority():
        gn_stats(0)
        aff0_mm = gn_affine(0, 1)
        xn_make(0, eng=nc.vector, skip_xnf=True)
    # second warm-up batch, pinned after affine(0)'s matmul: fills the
    # ~4us PE wait for xn8(0) at full clock.
    for i in range(14):
        wps = psum_big.tile([P, 512], F32, tag="big")
        m = nc.tensor.matmul(wps[:], ones2[:], dm8[:], start=True, stop=True,
                             perf_mode=DR)
        if i == 0:
            tile.add_dep_helper(m.ins, aff0_mm.ins,
                                reason="warmup2 after affine(0)")
    xnf_make(0)
    # modeled-ready floor: keep sample-1 stats out of the static engine
    # order until sample 0's chain has really run (x(1) lands ~22us real).
    with tc.tile_wait_until(0.015):
        gn_stats(1)
        gn_affine(1, 1)
    qk_phase(0)
    v_phase(0)
    xn_make(1)

    for s in range(SPC):
        scores_phase(s, split_exp=(s == SPC - 1))
        if s + 1 < SPC:
            qk_phase(s + 1)
            v_phase(s + 1)
        if s + 2 < SPC:
            x_dma(s + 2)
        softmax_phase(s)
        attnv_phase(s)
        proj_phase(s)
        if s + 2 < SPC:
            gn_stats(s + 2)
            gn_affine(s + 2, 1)
            xn_make(s + 2)


def build():
    nc = bacc.Bacc("TRN2", target_bir_lowering=False, debug=False)
    aps = _declare_io(nc)
    with tile.TileContext(nc) as tc:
        with ExitStack() as ctx:
            _build_tile_kernel(ctx, tc, aps)
    nc.compile()
    return nc


_cached_nc = None


def _get_nc():
    global _cached_nc
    if _cached_nc is None:
        _cached_nc = build()
    return _cached_nc


def _host_inputs(gamma, beta, Wq, bq, Wk, bk, Wv, bv, Wp, bp):
    import ml_dtypes
    f = lambda a: np.ascontiguousarray(np.asarray(a, dtype=np.float32))

    def wdr(Wmat):
        Wt = np.asarray(Wmat, np.float64).T * WS            # [in, out]
        arr = Wt.reshape(2, 2, P, C).transpose(2, 0, 1, 3)  # [p, j, i, m]
        return np.ascontiguousarray(
            arr.astype(np.float32).astype(ml_dtypes.float8_e4m3))

    member_t = np.zeros((2, P), np.float32)
    member_t[0, :GSZ] = 1.0
    member_t[1, GSZ:] = 1.0
    bp_eff = (np.asarray(bp, np.float64)
              + np.asarray(Wp, np.float64) @ np.asarray(bv, np.float64)
              ).astype(np.float32)
    smalls = np.zeros((P, 23), np.float32)
    for i, v in enumerate((bq, bk, bp_eff, gamma, beta)):
        smalls[:, i * CT:(i + 1) * CT] = f(v).reshape(CT, P).T
    smalls[:GSZ, 20] = 1.0 / GSZ
    smalls[GSZ:, 21] = 1.0 / GSZ
    smalls[:, 22] = 1.0
    return {
        "wq8": wdr(Wq), "wk8": wdr(Wk), "wv8": wdr(Wv), "wp8": wdr(Wp),
        "smalls": smalls, "member_t": member_t,
    }


def run(inputs, trace=False, **kw):
    """Returns (out [B,C,H,W], BassKernelResults)."""
    nc = _get_nc()
    x = np.ascontiguousarray(np.asarray(inputs["x"], np.float32)).reshape(B, C, HW)
    common = _host_inputs(**{k: v for k, v in inputs.items() if k != "x"})
    in_maps = [dict(common, x=x[c * SPC:(c + 1) * SPC]) for c in range(N_CORES)]
    res = run_bass_kernel_spmd(nc, in_maps, core_ids=list(range(N_CORES)),
                               trace=trace, **kw)
    out = np.concatenate([res.results[c]["out"] for c in range(N_CORES)], axis=0)
    return out.reshape(B, C, H, W), res


def kernel(**inputs):
    out, _ = run(inputs)
    return out


# revision 15
# speedup vs baseline: 1.1746x; 1.1746x over previous
"""AttentionBlock (GroupNorm + single-head self-attention + residual) on 8 TRN2 cores.

Data-parallel over batch: 32 samples -> 4 per core; weights replicated.

fp8e4 DoubleRow matmuls (2 K-tiles of 128 contracted per instruction, 2x PE
throughput vs bf16). All operands stored K-paired: tile j holds K blocks
2j (sub 0) and 2j+1 (sub 1) as [128, 2, free].

Weights are scaled x64 on host (raw weights ~U(-0.044, 0.044) sit in fp8's
subnormal range); the 1/64 descale folds into existing evict ops. The
fused rowsum matmul uses an all-(1/64) stationary so every psum partition
receives rowsum/64 (rowsum + broadcast + descale in one step); oT is then
~unit-scale in fp8 and the projection evict descales by 1/4096. bp (+
Wp@bv) is folded into the residual shift column (shbp).

Software pipeline (engine queues are in-order, so emission order is
execution order): sample s+1's Q/K/V matmuls are emitted between sample
s's scores and attention, hiding the ACT Exp latency; x is DMA'd once and
cached in SBUF (stats + xn read the same tiles); stats/affine for sample
s+2 are prefetched a round ahead.

Head choreography (the DMA rings saturate at ~300 GB/s and process
descriptors roughly FIFO): x(0)'s triggers are emitted first from three
queues; weights wait on a Pool-side barrier read of x(0), and x(1) is
triggered on the Pool queue behind the weights, so the ring order is
strictly x(0) | weights | x(1). Meanwhile PE runs a chain of warm-up
matmuls on dummy data: they fill the otherwise-idle head and ramp the PE
p-state (a PE gap drops the clock to half for the next ~3us) so the first
real matmuls run at full speed.
"""

import sys

if "/opt/trn_rl_repo" not in sys.path:
    sys.path.insert(0, "/opt/trn_rl_repo")

from contextlib import ExitStack

import numpy as np

import concourse.bass as bass
import concourse.tile as tile
from concourse import bacc, mybir
from concourse.bass_utils import run_bass_kernel_spmd

N_CORES = 8
B, C, H, W = 32, 512, 32, 32
HW = H * W            # tokens per sample (N)
SPC = B // N_CORES    # samples per core
G = 8                 # groups
GSZ = C // G          # channels per group (64)
EPS = 1e-5
P = 128               # partitions
CT = C // P           # channel tiles (4)
NT = HW // P          # token tiles (8)
NCHUNK = HW // 512    # 512-wide free-dim chunks over tokens (2)
SCALE = C ** -0.5
WS = 64.0             # host-side weight scale (fp8 range)
IWS = 1.0 / WS
N_WARMUP = 42         # PE warm-up matmuls during the head

F32 = mybir.dt.float32
FP8 = mybir.dt.float8e4
AF = mybir.ActivationFunctionType
ALU = mybir.AluOpType
DR = mybir.MatmulPerfMode.DoubleRow

FREE = 512   # moving-operand width per matmul (PSUM one-bank limit)
NJC = HW // FREE


def _declare_io(nc):
    def inp(name, shape, dt=F32):
        return nc.dram_tensor(name, list(shape), dt, kind="ExternalInput").ap()

    aps = {
        "x": inp("x", (SPC, C, HW)),
        "wq8": inp("wq8", (P, 2, 2, C), FP8),
        "wk8": inp("wk8", (P, 2, 2, C), FP8),
        "wv8": inp("wv8", (P, 2, 2, C), FP8),
        "wp8": inp("wp8", (P, 2, 2, C), FP8),
        "smalls": inp("smalls", (P, 23)),
        "member_t": inp("member_t", (2, P)),
        "out": nc.dram_tensor("out", [SPC, C, HW], F32, kind="ExternalOutput").ap(),
    }
    return aps


def _build_tile_kernel(ctx: ExitStack, tc: tile.TileContext, aps):
    nc = tc.nc

    singles = ctx.enter_context(tc.tile_pool(name="singles", bufs=1))
    wpool = ctx.enter_context(tc.tile_pool(name="wpool", bufs=1))
    xpool = ctx.enter_context(tc.tile_pool(name="xpool", bufs=2))
    xnpool = ctx.enter_context(tc.tile_pool(name="xnpool", bufs=2))
    qpool = ctx.enter_context(tc.tile_pool(name="qpool", bufs=2))
    kpool = ctx.enter_context(tc.tile_pool(name="kpool", bufs=2))
    vpool = ctx.enter_context(tc.tile_pool(name="vpool", bufs=2))
    epool = ctx.enter_context(tc.tile_pool(name="epool", bufs=2))
    opool = ctx.enter_context(tc.tile_pool(name="opool", bufs=2))
    stat = ctx.enter_context(tc.tile_pool(name="stat", bufs=2))
    sall = ctx.enter_context(tc.tile_pool(name="sall", bufs=1))
    rpool = ctx.enter_context(tc.tile_pool(name="rpool", bufs=2))
    respool = ctx.enter_context(tc.tile_pool(name="respool", bufs=5))

    psum_big = ctx.enter_context(tc.tile_pool(name="psum_big", bufs=3, space="PSUM"))
    psum_sm = ctx.enter_context(tc.tile_pool(name="psum_sm", bufs=2, space="PSUM"))

    xtiles = {}   # (s, ct) -> cached x tile (loaded once, reused by xn)

    def x_dma(s, spread=False, eng=None):
        # spread=True issues the 8 half-DMAs from 3 different engine queues
        # so descriptor generation (~650ns per trigger) parallelizes -- used
        # for sample 0 where the x load is the critical path. eng pins all
        # triggers to one queue (used to sequence x(1) behind the weights on
        # the Pool queue).
        # full-tile DMAs: 4KB descriptors halve ring occupancy vs 2KB halves
        engs = ([nc.sync, nc.scalar, nc.gpsimd] if spread
                else [eng or nc.sync])
        for ct in range(CT):
            xt = xpool.tile([P, HW], F32, name=f"x_{s}_{ct}", tag=f"x{ct}")
            engs[ct % len(engs)].dma_start(
                out=xt[:], in_=aps["x"][s, ct * P:(ct + 1) * P, :])
            xtiles[(s, ct)] = xt

    # ---- head: x(0) descriptors hit the rings before anything else ----
    x_dma(0, spread=True)

    # small constants (scalar queue; lands right behind x(0)'s first rows)
    smalls = singles.tile([P, 23], F32, tag="smalls")
    nc.scalar.dma_start(out=smalls[:], in_=aps["smalls"][:])
    bq_c = smalls[:, 0:CT]
    bk_c = smalls[:, CT:2 * CT]
    bp_c = smalls[:, 2 * CT:3 * CT]
    gamma_c = smalls[:, 3 * CT:4 * CT]
    beta_c = smalls[:, 4 * CT:5 * CT]
    member = smalls[:, 20:22]

    member_t = singles.tile([2, P], F32, tag="member_t")
    nc.scalar.dma_start(out=member_t[:], in_=aps["member_t"][:])

    # fp8 (1/WS) stationary for the fused rowsum+broadcast DR matmul, plus
    # a 512-wide dummy moving tile for the PE warm-up chain.
    ones_stage = singles.tile([P, HW], F32, tag="ones_stage")
    nc.vector.memset(ones_stage[:], 1.0 / WS)
    ones2 = singles.tile([P, 2, P], FP8, tag="ones2")
    nc.vector.tensor_copy(out=ones2[:].rearrange("p i m -> p (i m)"),
                          in_=ones_stage[:, 0:2 * P])
    dm8 = singles.tile([P, 2, 512], FP8, tag="dm8")
    nc.vector.tensor_copy(out=dm8[:].rearrange("p i m -> p (i m)"),
                          in_=ones_stage[:])

    I32 = mybir.dt.int32
    magic = singles.tile([2, SPC * CT], I32, tag="magic")
    nc.vector.memset(magic[:], 0x5F3759DF)

    # prime the ACT Exp spline table before any real dependency needs it
    warm = singles.tile([1, 1], F32, tag="warm")
    nc.vector.memset(warm[:], 1.0)
    nc.scalar.activation(out=warm[:], in_=warm[:], func=AF.Exp)

    # PE warm-up: fills the DMA/stats-bound head and ramps the PE p-state
    # so the first real matmuls run at full clock.
    for i in range(N_WARMUP):
        wps = psum_big.tile([P, 512], F32, tag="big")
        nc.tensor.matmul(wps[:], ones2[:], dm8[:], start=True, stop=True,
                         perf_mode=DR)

    # ---- weights: Pool-queue barrier behind x(0), then x(1) behind them ----
    wr = {}

    def load_weights(barrier_tile):
        if barrier_tile is not None:
            dummy = singles.tile([P, 1], F32, tag="wbar")
            nc.gpsimd.tensor_copy(out=dummy[:], in_=barrier_tile[:, 1023:1024])
        for wname in ("wq8", "wk8", "wv8", "wp8"):
            t = wpool.tile([P, 2, 2, C], FP8, name=f"{wname}t", tag=wname)
            nc.gpsimd.dma_start(out=t[:], in_=aps[wname][:])
            wr[wname] = t

    load_weights(xtiles[(0, 3)])
    x_dma(1, eng=nc.gpsimd)

    def w_sl(wname, j, blk):
        return wr[wname][:, j, :, blk * P:(blk + 1) * P]

    # ======== GroupNorm statistics ========
    stats_all = sall.tile([2, SPC, CT, 2], F32, tag="stats_all")
    sc = sall.tile([P, SPC, CT], F32, tag="sc")
    sh = sall.tile([P, SPC, CT], F32, tag="sh")
    shbp = sall.tile([P, SPC, CT], F32, tag="shbp")

    def gn_stats(s, pe_after=None, dve_after=None):
        # Anchors pin this block behind the given instructions in the static
        # engine queues -- the build-time scheduler otherwise hoists it into
        # earlier PE/DVE slots (its DMA model is optimistic about x's
        # arrival) and the real hardware stalls there.
        partials = stat.tile([P, CT, 2], F32, tag="partials")
        for ct in range(CT):
            xt = xtiles[(s, ct)]
            st6 = stat.tile([P, 2, 6], F32, tag="st6")
            d = nc.vector.bn_stats(out=st6[:, 0, :], in_=xt[:, 0:512])
            if dve_after is not None:
                tile.add_dep_helper(d.ins, dve_after.ins,
                                    reason="stats yield DVE to evicts")
                dve_after = None
            nc.vector.bn_stats(out=st6[:, 1, :], in_=xt[:, 512:1024])
            nc.vector.bn_aggr(out=partials[:, ct, :], in_=st6[:])
            nc.vector.scalar_tensor_tensor(
                out=partials[:, ct, 1:2], in0=partials[:, ct, 0:1],
                scalar=partials[:, ct, 0:1], in1=partials[:, ct, 1:2],
                op0=ALU.mult, op1=ALU.add)
        ps = psum_sm.tile([2, CT * 2], F32, tag="sm")
        m = nc.tensor.matmul(ps[:], member[:],
                             partials[:].rearrange("p t j -> p (t j)"),
                             start=True, stop=True)
        if pe_after is not None:
            tile.add_dep_helper(m.ins, pe_after.ins,
                                reason="stats-mm after attnV stream")
        nc.vector.tensor_copy(out=stats_all[:, s, :, :],
                              in_=ps[:].rearrange("p (t j) -> p t j", j=2))

    def gn_affine(s0, ns):
        mv = stats_all[:, s0:s0 + ns, :, 0]
        sv = stats_all[:, s0:s0 + ns, :, 1]
        msq = stat.tile([2, ns, CT], F32, tag="msq")
        nc.vector.tensor_mul(out=msq[:], in0=mv, in1=mv)
        nc.vector.tensor_sub(out=sv, in0=sv, in1=msq[:])
        # rstd = rsqrt(var + eps) on DVE: bit-trick seed + 3 Newton iters.
        vadd = stat.tile([2, ns, CT], F32, tag="vadd")
        nc.vector.tensor_scalar_add(out=vadd[:], in0=sv, scalar1=float(EPS))
        z = stat.tile([2, ns, CT], F32, tag="z")
        z_i = z[:].bitcast(I32)
        nc.vector.tensor_scalar(out=z_i, in0=vadd[:].bitcast(I32), scalar1=1,
                                scalar2=None, op0=ALU.arith_shift_right)
        mg = magic[:, 0:ns * CT].rearrange("p (s t) -> p s t", t=CT)
        nc.vector.scalar_tensor_tensor(out=z_i, in0=mg, scalar=0, in1=z_i,
                                       op0=ALU.bypass, op1=ALU.subtract)
        # 2 Newton iterations: bit-trick seed err ~3.4% -> 0.17% -> 4e-6,
        # negligible vs the fp8 path's ~1e-3.
        nt_ = stat.tile([2, ns, CT], F32, tag="nt")
        for _ in range(2):
            nc.vector.tensor_mul(out=nt_[:], in0=z[:], in1=z[:])
            nc.vector.tensor_mul(out=nt_[:], in0=nt_[:], in1=vadd[:])
            nc.vector.tensor_scalar(out=nt_[:], in0=nt_[:], scalar1=-0.5,
                                    scalar2=1.5, op0=ALU.mult, op1=ALU.add)
            nc.vector.tensor_mul(out=z[:], in0=z[:], in1=nt_[:])
        nc.vector.tensor_copy(out=sv, in_=z[:])
        ab = stat.tile([2, ns, CT, 2], F32, tag="ab")
        nc.vector.tensor_copy(out=ab[:, :, :, 0], in_=sv)
        nc.vector.scalar_tensor_tensor(out=ab[:, :, :, 1], in0=mv, scalar=-1.0,
                                       in1=sv, op0=ALU.mult, op1=ALU.mult)
        sb_ps = psum_sm.tile([P, ns * CT * 2], F32, tag="sm")
        sb_mm = nc.tensor.matmul(sb_ps[:], member_t[:],
                                 ab[:].rearrange("p s t j -> p (s t j)"),
                                 start=True, stop=True)
        sb = stat.tile([P, ns, CT, 2], F32, tag="sb")
        nc.vector.tensor_copy(
            out=sb[:], in_=sb_ps[:].rearrange("p (s t j) -> p s t j", t=CT, j=2))
        for i in range(ns):
            s = s0 + i
            for ct in range(CT):
                nc.vector.tensor_scalar_mul(out=sc[:, s, ct:ct + 1],
                                            in0=gamma_c[:, ct:ct + 1],
                                            scalar1=sb[:, i, ct, 0:1])
                nc.vector.scalar_tensor_tensor(out=sh[:, s, ct:ct + 1],
                                               in0=gamma_c[:, ct:ct + 1],
                                               scalar=sb[:, i, ct, 1:2],
                                               in1=beta_c[:, ct:ct + 1],
                                               op0=ALU.mult, op1=ALU.add)
                nc.vector.tensor_tensor(out=shbp[:, s, ct:ct + 1],
                                        in0=sh[:, s, ct:ct + 1],
                                        in1=bp_c[:, ct:ct + 1], op=ALU.add)
        return sb_mm

    # ======== per-sample phase emitters ========
    xn8_all = {}
    xnf_all = {}
    q8_all = {}
    k8_all = {}
    vT8_all = {}
    eT8_all = {}
    rb_all = {}
    oT8_all = {}

    def xn_make(s, eng=None, skip_xnf=False):
        # Pool (SBUF->SBUF): fp8 matmul input first (feeds PE), then the
        # f32 residual (only needed at the projection evict). Sample 0 uses
        # DVE (faster) since the head's first matmuls wait on it.
        eng = eng or nc.gpsimd
        xn8 = [xnpool.tile([P, 2, HW], FP8, name=f"xn8_{s}_{j}", tag=f"xn8{j}")
               for j in range(2)]
        last = None
        for ct in range(CT):
            last = eng.tensor_scalar(out=xn8[ct // 2][:, ct % 2, :],
                                     in0=xtiles[(s, ct)][:],
                                     scalar1=sc[:, s, ct:ct + 1],
                                     scalar2=sh[:, s, ct:ct + 1],
                                     op0=ALU.mult, op1=ALU.add)
        xn8_all[s] = xn8
        if not skip_xnf:
            xnf_make(s)
        return last

    def xnf_make(s):
        xnf = []
        for ct in range(CT):
            tf = xnpool.tile([P, HW], F32, name=f"xnf_{s}_{ct}", tag=f"xnf{ct}")
            nc.gpsimd.tensor_scalar(out=tf[:], in0=xtiles[(s, ct)][:],
                                    scalar1=sc[:, s, ct:ct + 1],
                                    scalar2=shbp[:, s, ct:ct + 1],
                                    op0=ALU.mult, op1=ALU.add)
            xnf.append(tf)
        xnf_all[s] = xnf

    def qk_phase(s):
        xn8 = xn8_all[s]
        last = None
        for pname, bcol, pool_, store in (
                ("q", bq_c, qpool, q8_all), ("k", bk_c, kpool, k8_all)):
            wname = "wq8" if pname == "q" else "wk8"
            tiles = [pool_.tile([P, 2, HW], FP8, name=f"{pname}8_{s}_{j}",
                                tag=f"{pname}{j}") for j in range(2)]
            for dt in range(CT):
                ps = psum_big.tile([P, HW], F32, tag="big")
                for jc in range(NJC):
                    for j in range(2):
                        last = nc.tensor.matmul(
                            ps[:, jc * FREE:(jc + 1) * FREE],
                            w_sl(wname, j, dt),
                            xn8[j][:, :, jc * FREE:(jc + 1) * FREE],
                            start=(j == 0), stop=(j == 1), perf_mode=DR)
                nc.scalar.activation(out=tiles[dt // 2][:, dt % 2, :], in_=ps[:],
                                     func=AF.Identity, bias=bcol[:, dt:dt + 1],
                                     scale=IWS)
            store[s] = tiles
        return last

    def v_phase(s, hold=0):
        # hold>0 defers the last `hold` token-groups to v_phase_tail -- used
        # as PE filler between attnV(s-1) and proj(s-1) to cover the
        # trailing oT evict latency.
        xn8 = xn8_all[s]
        vT8 = [vpool.tile([P, 2, C], FP8, name=f"vT8_{s}_{j}", tag=f"v{j}")
               for j in range(4)]
        vT8_all[s] = vT8
        for nt in range(NT - hold):
            _v_group(s, nt)

    def _v_group(s, nt):
        xn8, vT8 = xn8_all[s], vT8_all[s]
        ps_full = psum_big.tile([P, HW], F32, tag="big")
        ps = ps_full[:, 0:512]
        for j in range(2):
            nc.tensor.matmul(ps[:], xn8[j][:, :, nt * P:(nt + 1) * P],
                             wr["wv8"][:, j],
                             start=(j == 0), stop=(j == 1), perf_mode=DR)
        nc.scalar.activation(out=vT8[nt // 2][:, nt % 2, :], in_=ps[:],
                             func=AF.Identity, scale=IWS)

    def v_phase_tail(s, hold):
        for nt in range(NT - hold, NT):
            _v_group(s, nt)

    def scores_phase(s, split_exp=False):
        q8, k8 = q8_all[s], k8_all[s]
        eT8 = [epool.tile([P, 2, HW], FP8, name=f"eT8_{s}_{j}", tag=f"e{j}")
               for j in range(4)]
        pss = []
        for mt in range(NT):
            ps = psum_big.tile([P, HW], F32, tag="big")
            for jc in range(NJC):
                for j in range(2):
                    nc.tensor.matmul(ps[:, jc * FREE:(jc + 1) * FREE],
                                     k8[j][:, :, mt * P:(mt + 1) * P],
                                     q8[j][:, :, jc * FREE:(jc + 1) * FREE],
                                     start=(j == 0), stop=(j == 1), perf_mode=DR)
            if split_exp:
                # jc0-half exps first so the tail's rowsum/attnV jc0 groups
                # start ~5us earlier (used for the last sample).
                pss.append(ps)
                nc.scalar.activation(out=eT8[mt // 2][:, mt % 2, 0:512],
                                     in_=ps[:, 0:512], func=AF.Exp, scale=SCALE)
            else:
                nc.scalar.activation(out=eT8[mt // 2][:, mt % 2, :], in_=ps[:],
                                     func=AF.Exp, scale=SCALE)
        if split_exp:
            for mt in range(NT):
                nc.scalar.activation(out=eT8[mt // 2][:, mt % 2, 512:1024],
                                     in_=pss[mt][:, 512:1024], func=AF.Exp,
                                     scale=SCALE)
        eT8_all[s] = eT8

    def softmax_phase(s):
        # fused rowsum + broadcast + 1/WS: rb = WS/rowsum on every partition
        eT8 = eT8_all[s]
        rb = rpool.tile([P, HW], F32, name=f"rb_{s}", tag="rb")
        for jc in range(NCHUNK):
            rs_ps = psum_sm.tile([P, 512], F32, tag="sm")
            for j4 in range(4):
                nc.tensor.matmul(rs_ps[:], ones2[:],
                                 eT8[j4][:, :, jc * 512:(jc + 1) * 512],
                                 start=(j4 == 0), stop=(j4 == 3), perf_mode=DR)
            nc.vector.reciprocal_approx_fast(out=rb[:, jc * 512:(jc + 1) * 512],
                                             in_=rs_ps[:])
        rb_all[s] = rb

    def attnv_phase(s):
        eT8, vT8, rb = eT8_all[s], vT8_all[s], rb_all[s]
        oT8 = [opool.tile([P, 2, HW], FP8, name=f"oT8_{s}_{j}", tag=f"o{j}")
               for j in range(2)]
        # jc-outer: all jc0 evicts land first, so the projection's jc0
        # groups can run while the jc1 half is still evicting.
        last_mm = last_ev = None
        for jc in range(NCHUNK):
            for dt in range(CT):
                ps = psum_big.tile([P, 512], F32, tag="big")
                for j4 in range(4):
                    last_mm = nc.tensor.matmul(
                        ps[:], vT8[j4][:, :, dt * P:(dt + 1) * P],
                        eT8[j4][:, :, jc * 512:(jc + 1) * 512],
                        start=(j4 == 0), stop=(j4 == 3), perf_mode=DR)
                last_ev = nc.vector.tensor_mul(
                    out=oT8[dt // 2][:, dt % 2, jc * 512:(jc + 1) * 512],
                    in0=ps[:], in1=rb[:, jc * 512:(jc + 1) * 512])
        oT8_all[s] = oT8
        return last_mm, last_ev

    def proj_phase(s):
        # jc-outer to pair with attnv_phase: the jc0 projection only needs
        # the jc0 oT evicts (first half of attnV's evict stream).
        oT8, xnf = oT8_all[s], xnf_all[s]
        res_t = [respool.tile([P, HW], F32, name=f"res_{s}_{et}", tag="res")
                 for et in range(CT)]
        for jc in range(NJC):
            sl = slice(jc * FREE, (jc + 1) * FREE)
            for et in range(CT):
                ps = psum_big.tile([P, FREE], F32, tag="big")
                for j in range(2):
                    nc.tensor.matmul(ps[:], w_sl("wp8", j, et),
                                     oT8[j][:, :, sl],
                                     start=(j == 0), stop=(j == 1), perf_mode=DR)
                nc.vector.scalar_tensor_tensor(
                    out=res_t[et][:, sl], in0=ps[:], scalar=1.0 / (WS * WS),
                    in1=xnf[et][:, sl], op0=ALU.mult, op1=ALU.add)
                nc.sync.dma_start(out=aps["out"][s, et * P:(et + 1) * P, sl],
                                  in_=res_t[et][:, sl])

    # ======== schedule ========
    # The head's serial chain (x(0) -> stats -> affine -> xn8) gates the
    # first real matmul; high_priority keeps the build-time scheduler from
    # diluting it with later-emitted ready work (its DMA model is
    # optimistic, so sample-1 stats often look "ready" too early).
    with tc.high_priority():
        gn_stats(0)
        aff0_mm = gn_affine(0, 1)
        xn0_last = xn_make(0, eng=nc.vector, skip_xnf=True)
    # second warm-up batch, pinned after affine(0)'s matmul: fills the
    # ~4us PE wait for xn8(0) at full clock.
    for i in range(14):
        wps = psum_big.tile([P, 512], F32, tag="big")
        m = nc.tensor.matmul(wps[:], ones2[:], dm8[:], start=True, stop=True,
                             perf_mode=DR)
        if i == 0:
            tile.add_dep_helper(m.ins, aff0_mm.ins,
                                reason="warmup2 after affine(0)")
    xnf_make(0)
    qk0_last = qk_phase(0)
    # sample-1 stats: hard anchors (bn_stats after xn8(0) on DVE, its tiny
    # matmul after qk(0) on PE) keep the static queues from interleaving it
    # into sample 0's critical chain, where it would block on x(1)'s DMA.
    gn_stats(1, pe_after=qk0_last, dve_after=xn0_last)
    gn_affine(1, 1)
    v_phase(0)
    xn_make(1)

    for s in range(SPC):
        scores_phase(s, split_exp=(s == SPC - 1))
        if s + 1 < SPC:
            qk_phase(s + 1)
            v_phase(s + 1)
        if s + 2 < SPC:
            x_dma(s + 2)
        softmax_phase(s)
        attnv_phase(s)
        proj_phase(s)
        if s + 2 < SPC:
            gn_stats(s + 2)
            gn_affine(s + 2, 1)
            xn_make(s + 2)


def build():
    nc = bacc.Bacc("TRN2", target_bir_lowering=False, debug=False)
    aps = _declare_io(nc)
    with tile.TileContext(nc) as tc:
        with ExitStack() as ctx:
            _build_tile_kernel(ctx, tc, aps)
    nc.compile()
    return nc


_cached_nc = None


def _get_nc():
    global _cached_nc
    if _cached_nc is None:
        _cached_nc = build()
    return _cached_nc


def _host_inputs(gamma, beta, Wq, bq, Wk, bk, Wv, bv, Wp, bp):
    import ml_dtypes
    f = lambda a: np.ascontiguousarray(np.asarray(a, dtype=np.float32))

    def wdr(Wmat):
        Wt = np.asarray(Wmat, np.float64).T * WS            # [in, out]
        arr = Wt.reshape(2, 2, P, C).transpose(2, 0, 1, 3)  # [p, j, i, m]
        return np.ascontiguousarray(
            arr.astype(np.float32).astype(ml_dtypes.float8_e4m3))

    member_t = np.zeros((2, P), np.float32)
    member_t[0, :GSZ] = 1.0
    member_t[1, GSZ:] = 1.0
    bp_eff = (np.asarray(bp, np.float64)
              + np.asarray(Wp, np.float64) @ np.asarray(bv, np.float64)
              ).astype(np.float32)
    smalls = np.zeros((P, 23), np.float32)
    for i, v in enumerate((bq, bk, bp_eff, gamma, beta)):
        smalls[:, i * CT:(i + 1) * CT] = f(v).reshape(CT, P).T
    smalls[:GSZ, 20] = 1.0 / GSZ
    smalls[GSZ:, 21] = 1.0 / GSZ
    smalls[:, 22] = 1.0
    return {
        "wq8": wdr(Wq), "wk8": wdr(Wk), "wv8": wdr(Wv), "wp8": wdr(Wp),
        "smalls": smalls, "member_t": member_t,
    }


def run(inputs, trace=False, **kw):
    """Returns (out [B,C,H,W], BassKernelResults)."""
    nc = _get_nc()
    x = np.ascontiguousarray(np.asarray(inputs["x"], np.float32)).reshape(B, C, HW)
    common = _host_inputs(**{k: v for k, v in inputs.items() if k != "x"})
    in_maps = [dict(common, x=x[c * SPC:(c + 1) * SPC]) for c in range(N_CORES)]
    res = run_bass_kernel_spmd(nc, in_maps, core_ids=list(range(N_CORES)),
                               trace=trace, **kw)
    out = np.concatenate([res.results[c]["out"] for c in range(N_CORES)], axis=0)
    return out.reshape(B, C, H, W), res


def kernel(**inputs):
    out, _ = run(inputs)
    return out
